# revision 2
# baseline (speedup 1.0000x reference)
"""Trainium2 Bass kernel for BipartiteGNNConvFactorToVariable (plan D).

  out = variables + relu(concat([variables, aggr]) @ W_comb + b_comb)
  aggr = segment_sum(relu(concat([x_i, x_j, 0]) @ W_msg + b_msg), v_to_f)
  x_i = variables[v_to_f], x_j = factors[f_to_v]

Distribution (8 cores, zero collectives): host packs variables into 128-slot
blocks balanced by edge degree (98 blocks/core, LPT snake-deal); every edge is
assigned an edge slot of its target variable's block, so the segment-sum is
fully core-local.  CAP=1280 edge slots per block (10 tiles of 128).

Device-side per 128-edge tile (all fp8 = e3m4, W pre-scaled x32 on host,
un-scaled in the relu):
  m  = relu((xiT.T@W1' + xjT.T@W2')/32)   two e3m4 matmuls
  S  = is_equal(vtf, iota)   batched per block on vector (bf16)
  aggrT += m.T @ S           bf16 matmul per tile
Per 4 blocks (transposed combine, all bf16, Wc stationary, N=512):
  hT = relu(Wc1.T@V_T + Wc2.T@aggrT [+ b_comb per-partition])
  outT = V_T + hT   ->  DRAM [D, NVC] bf16, host transposes back.

The host pre-gathers x_i/x_j per edge slot into ONE fp8 stream
xp[d, chunk, 0|1, e] = xiT|xjT (device-side indirect gather is unusable in
this toolchain), and ships V^T once (bf16, SBUF-resident).
"""

import numpy as np
import ml_dtypes

import concourse.bass as bass
import concourse.tile as tile
from concourse import mybir
from concourse.bass_utils import run_bass_kernel_spmd

BF16 = ml_dtypes.bfloat16
FP8 = ml_dtypes.float8_e3m4
W_SCALE = 32.0

NV, NF, E, D = 100000, 50000, 1000000, 128
NC = 8
NBLK_CORE = 98              # blocks per core
NBLK = NC * NBLK_CORE       # 784
NVC = NBLK_CORE * 128       # 12544 variable slots per core
CAP = 1280                  # edge slots per block (10 tiles)
SGROUP = 16                 # blocks per staging group
CGROUP = 4                  # blocks per combine group


def pack_blocks(v_to_f):
    """Assign variables to (block, slot) with balanced per-block degree."""
    deg = np.bincount(v_to_f, minlength=NV).astype(np.int64)
    vids = np.argsort(-deg, kind="stable")
    blk_load = np.zeros(NBLK, np.int64)
    blk_of = np.full(NV, -1, np.int32)
    for r in range(128):
        chunk = vids[r * NBLK:(r + 1) * NBLK]
        order_blocks = np.argsort(blk_load, kind="stable")
        blk_of[chunk] = order_blocks[: len(chunk)]
        np.add.at(blk_load, order_blocks[: len(chunk)], deg[chunk])

    order = np.lexsort((np.arange(NV), blk_of))
    slot_of = np.empty(NV, np.int32)
    counts = np.bincount(blk_of, minlength=NBLK)
    starts = np.concatenate([[0], np.cumsum(counts)[:-1]])
    slot_of[order] = (np.arange(NV) - starts[blk_of[order]]).astype(np.int32)

    vid_of = np.full((NBLK, 128), -1, np.int64)
    vid_of[blk_of, slot_of] = np.arange(NV)
    return blk_of, slot_of, vid_of, int(blk_load.max())


def build_host_data(variables, factors, v_to_f, f_to_v,
                    W_msg, b_msg, W_comb, b_comb, cap):
    T = cap // 128
    nslots = NBLK_CORE * cap
    nchunk = NBLK_CORE * T
    blk_of, slot_of, vid_of, max_deg = pack_blocks(v_to_f)
    assert max_deg <= cap, max_deg

    eblk = blk_of[v_to_f]
    order = np.argsort(eblk, kind="stable")
    counts = np.bincount(eblk, minlength=NBLK)
    starts = np.concatenate([[0], np.cumsum(counts)[:-1]])
    rank = np.arange(E) - starts[eblk[order]]

    core_e = (eblk[order] // NBLK_CORE).astype(np.int64)
    pos = (eblk[order] % NBLK_CORE) * cap + rank

    variables_f8 = variables.astype(FP8)
    factors_f8 = factors.astype(FP8)

    wp = np.empty((D, 2, D), FP8)
    wp[:, 0, :] = (W_msg[0:D] * W_SCALE).astype(FP8)
    wp[:, 1, :] = (W_msg[D:2 * D] * W_SCALE).astype(FP8)

    in_maps = []
    for c in range(NC):
        sel = core_e == c
        posc = pos[sel]
        ec = order[sel]
        xiT = np.zeros((D, nslots), FP8)
        xjT = np.zeros((D, nslots), FP8)
        xiT[:, posc] = variables_f8[v_to_f[ec]].T
        xjT[:, posc] = factors_f8[f_to_v[ec]].T
        xp = np.empty((D, nchunk, 2, 128), FP8)
        xp[:, :, 0, :] = xiT.reshape(D, nchunk, 128)
        xp[:, :, 1, :] = xjT.reshape(D, nchunk, 128)
        vt = np.full(nslots, -1.0, np.float32)
        vt[posc] = slot_of[v_to_f[ec]].astype(np.float32)

        vids = vid_of[c * NBLK_CORE:(c + 1) * NBLK_CORE].reshape(-1)
        mask = vids >= 0
        vperm = np.zeros((NVC, D), np.float32)
        vperm[mask] = variables[vids[mask]]

        in_maps.append(dict(
            xp=np.ascontiguousarray(xp.reshape(D, nchunk * 256)),
            vtf=np.ascontiguousarray(
                vt.reshape(nchunk, 128).T.astype(BF16)),
            vpermT=np.ascontiguousarray(vperm.T.astype(BF16)),
            wp=np.ascontiguousarray(wp.reshape(D, 2 * D)),
            wc1=np.ascontiguousarray(W_comb[0:D]).astype(BF16),
            wc2=np.ascontiguousarray(W_comb[D:2 * D]).astype(BF16),
            iota_bf=np.ascontiguousarray(np.broadcast_to(
                np.arange(D, dtype=np.float32), (128, D)).astype(BF16)),
        ))

    has_msg_bias = bool(np.any(b_msg != 0))
    has_comb_bias = bool(np.any(b_comb != 0))
    if has_msg_bias:
        for m in in_maps:
            m["bmsg_f8"] = (b_msg.reshape(1, D) * W_SCALE).astype(FP8)
            m["ones_f8"] = np.ones((1, D), FP8)
    if has_comb_bias:
        for m in in_maps:
            m["bcomb32"] = b_comb.reshape(D, 1).astype(np.float32)
    return in_maps, vid_of, has_msg_bias, has_comb_bias


def split_multi_waits(nc, max_waits=1):
    """This walrus rejects >1 sync-wait command on an instruction; move the
    extras onto injected NoOps just before it (same engine, program order)."""
    for fn in nc.m.functions:
        for bb in fn.blocks:
            new_insts = []
            for inst in bb.instructions:
                si = inst.sync_info
                if (si is not None and si.on_wait
                        and len(si.on_wait) > max_waits):
                    waits = list(si.on_wait)
                    move, keep = waits[:-max_waits], waits[-max_waits:]
                    for j, w in enumerate(move):
                        nop = mybir.InstNoOp(
                            name=f"{inst.name}-wsplit{j}",
                            sync_info=mybir.SyncInfo(on_wait=[w],
                                                     on_update=[]),
                            bass_nofuse=True,
                            engine=inst.engine,
                        )
                        nc.register_instruction(nop)
                        new_insts.append(nop)
                    si.on_wait = keep
                new_insts.append(inst)
            bb.instructions[:] = new_insts
    return nc


def build_nc(cap, has_msg_bias, has_comb_bias):
    T = cap // 128
    NCHUNK = NBLK_CORE * T
    DR = mybir.MatmulPerfMode.DoubleRow

    f32, bf, f8 = mybir.dt.float32, mybir.dt.bfloat16, mybir.dt.float8e3
    nc = bass.Bass("TRN2", target_bir_lowering=False, debug=False,
                   num_devices=NC)

    xp_d = nc.dram_tensor("xp", [D, NCHUNK * 256], f8,
                          kind="ExternalInput").ap()
    vtf_d = nc.dram_tensor("vtf", [128, NCHUNK], bf,
                           kind="ExternalInput").ap()
    vpermT_d = nc.dram_tensor("vpermT", [D, NVC], bf,
                              kind="ExternalInput").ap()
    wp_d = nc.dram_tensor("wp", [D, 2 * D], f8, kind="ExternalInput").ap()
    wc1_d = nc.dram_tensor("wc1", [D, D], bf, kind="ExternalInput").ap()
    wc2_d = nc.dram_tensor("wc2", [D, D], bf, kind="ExternalInput").ap()
    iota_d = nc.dram_tensor("iota_bf", [128, D], bf,
                            kind="ExternalInput").ap()
    if has_msg_bias:
        bmsg_d = nc.dram_tensor("bmsg_f8", [1, D], f8,
                                kind="ExternalInput").ap()
        ones_d = nc.dram_tensor("ones_f8", [1, D], f8,
                                kind="ExternalInput").ap()
    if has_comb_bias:
        bcomb_d = nc.dram_tensor("bcomb32", [D, 1], f32,
                                 kind="ExternalInput").ap()
    out_d = nc.dram_tensor("out", [D, NVC], bf, kind="ExternalOutput").ap()

    s_build = {"gpsimd": None, "vector": None}  # filled inside ctx

    with tile.TileContext(nc) as tc:
        with (tc.tile_pool(name="const", bufs=1) as constp,
              tc.tile_pool(name="stage", bufs=2) as stagep,
              tc.tile_pool(name="sblk", bufs=2) as sblkp,
              tc.tile_pool(name="mw", bufs=4) as mwp,
              tc.tile_pool(name="cw", bufs=2) as cwp,
              tc.tile_pool(name="psum_m", bufs=3, space="PSUM") as psmp,
              tc.tile_pool(name="psum_a", bufs=2, space="PSUM") as psap,
              tc.tile_pool(name="psum_h", bufs=2, space="PSUM") as pshp):

            wp_s = constp.tile([128, 2, 128], f8)
            nc.sync.dma_start(wp_s[:], wp_d.rearrange(
                "p (two f) -> p two f", two=2))
            wc1_s = constp.tile([128, 128], bf)
            nc.sync.dma_start(wc1_s[:], wc1_d[:])
            wc2_s = constp.tile([128, 128], bf)
            nc.sync.dma_start(wc2_s[:], wc2_d[:])
            iota_s = constp.tile([128, 128], bf)
            nc.sync.dma_start(iota_s[:], iota_d[:])
            vtf_s = constp.tile([128, NCHUNK], bf)
            nc.sync.dma_start(vtf_s[:], vtf_d[:])
            vpermT_s = constp.tile([128, NVC], bf)
            nc.sync.dma_start(vpermT_s[:], vpermT_d[:])
            if has_msg_bias:
                bmsg_s = constp.tile([1, 128], f8)
                nc.sync.dma_start(bmsg_s[:], bmsg_d[:])
                ones_s = constp.tile([1, 128], f8)
                nc.sync.dma_start(ones_s[:], ones_d[:])
            if has_comb_bias:
                bcomb_s = constp.tile([128, 1], f32)
                nc.sync.dma_start(bcomb_s[:], bcomb_d[:])

            nblk_done = 0
            while nblk_done < NBLK_CORE:
                gnb = min(SGROUP, NBLK_CORE - nblk_done)
                c0 = nblk_done * T
                nch = gnb * T

                xp_st = stagep.tile([128, SGROUP * T, 2, 128], f8,
                                    tag="xp_st")
                nc.sync.dma_start(
                    xp_st[:, :nch, :, :],
                    xp_d[:, c0 * 256:(c0 + nch) * 256].rearrange(
                        "p (t two f) -> p t two f", two=2, f=128))

                cb = 0
                while cb < gnb:
                    cnb = min(CGROUP, gnb - cb)
                    pa4 = psap.tile([128, CGROUP, 128], f32, tag="pa")
                    for b in range(cnb):
                        blk = nblk_done + cb + b
                        # S for the whole block: [e, tile, v] (bf16)
                        S_s = sblkp.tile([128, T, 128], bf, tag="S")
                        nc.vector.tensor_tensor(
                            S_s[:],
                            vtf_s[:, blk * T:(blk + 1) * T]
                            .unsqueeze(2).broadcast_to((128, T, 128)),
                            iota_s[:].unsqueeze(1)
                            .broadcast_to((128, T, 128)),
                            op=mybir.AluOpType.is_equal)

                        for grp4 in range((T + 3) // 4):
                            t0 = grp4 * 4
                            nt = min(4, T - t0)
                            pm = psmp.tile([128, 4, 128], f32, tag="pm")
                            for t in range(t0, t0 + nt):
                                cc = (cb + b) * T + t
                                nc.tensor.matmul(
                                    pm[:, t - t0, :],
                                    xp_st[:, cc, 0, :], wp_s[:, 0, :],
                                    start=True, stop=False)
                                nc.tensor.matmul(
                                    pm[:, t - t0, :],
                                    xp_st[:, cc, 1, :], wp_s[:, 1, :],
                                    start=False, stop=not has_msg_bias)
                                if has_msg_bias:
                                    nc.tensor.matmul(
                                        pm[:, t - t0, :], ones_s[:],
                                        bmsg_s[:], start=False, stop=True)
                            m_s = mwp.tile([128, 4, 128], bf, tag="m")
                            nc.scalar.activation(
                                m_s[:, :nt, :], pm[:, :nt, :],
                                mybir.ActivationFunctionType.Relu,
                                scale=1.0 / W_SCALE)
                            for t in range(t0, t0 + nt):
                                nc.tensor.matmul(
                                    pa4[:, b, :],
                                    m_s[:, t - t0, :],
                                    S_s[:, t, :],
                                    start=(t == 0), stop=(t == T - 1))

                    # combine for cnb blocks (transposed, bf16, N=cnb*128)
                    g0 = nblk_done + cb
                    w = cnb * 128
                    ag4 = cwp.tile([128, CGROUP * 128], bf, tag="ag4")
                    nc.vector.tensor_copy(
                        ag4[:, :w],
                        pa4[:, :cnb, :].rearrange("p a b -> p (a b)"))
                    ph4 = pshp.tile([128, CGROUP * 128], f32, tag="ph4")
                    nc.tensor.matmul(ph4[:, :w], wc1_s[:],
                                     vpermT_s[:, g0 * 128:g0 * 128 + w],
                                     start=True, stop=False)
                    nc.tensor.matmul(ph4[:, :w], wc2_s[:], ag4[:, :w],
                                     start=False, stop=True)
                    h4 = cwp.tile([128, CGROUP * 128], bf, tag="h4")
                    nc.scalar.activation(
                        h4[:, :w], ph4[:, :w],
                        mybir.ActivationFunctionType.Relu,
                        bias=bcomb_s[:] if has_comb_bias else 0.0)
                    o4 = cwp.tile([128, CGROUP * 128], bf, tag="o4")
                    nc.vector.tensor_tensor(
                        o4[:, :w], h4[:, :w],
                        vpermT_s[:, g0 * 128:g0 * 128 + w],
                        op=mybir.AluOpType.add)
                    nc.sync.dma_start(
                        out_d[:, g0 * 128:g0 * 128 + w], o4[:, :w])
                    cb += cnb
                nblk_done += gnb

    split_multi_waits(nc)
    return nc


_RUN_KW = {}   # test harness can inject run_bass_kernel_spmd kwargs


def kernel(variables, factors, v_to_f, f_to_v, edge_attr,
           W_msg, b_msg, W_comb, b_comb):
    variables = np.asarray(variables, np.float32)
    factors = np.asarray(factors, np.float32)
    v_to_f = np.asarray(v_to_f, np.int32)
    f_to_v = np.asarray(f_to_v, np.int32)
    W_msg = np.asarray(W_msg, np.float32)
    b_msg = np.asarray(b_msg, np.float32)
    W_comb = np.asarray(W_comb, np.float32)
    b_comb = np.asarray(b_comb, np.float32)

    cap = CAP
    while True:
        try:
            in_maps, vid_of, has_mb, has_cb = build_host_data(
                variables, factors, v_to_f, f_to_v,
                W_msg, b_msg, W_comb, b_comb, cap)
            break
        except AssertionError:
            cap += 256  # keep T even (DR scatter consumes tile pairs)

    nc = build_nc(cap, has_mb, has_cb)
    res = run_bass_kernel_spmd(nc, in_maps, list(range(NC)), **_RUN_KW)

    out_full = np.zeros((NV, D), np.float32)
    for c in range(NC):
        vids = vid_of[c * NBLK_CORE:(c + 1) * NBLK_CORE].reshape(-1)
        mask = vids >= 0
        oc = np.asarray(res.results[c]["out"]).T.astype(np.float32)
        out_full[vids[mask]] = oc[mask]
    kernel.last_results = res
    return out_full
